# revision 19
# baseline (speedup 1.0000x reference)
"""EquiAttention Trainium2 kernel (v5: pure ST->exp->PV device pipeline).

Computes the reference nn_EquiAttention forward pass on 8 NeuronCores,
data-parallel over the batch axis (64 batches -> 8 per core).

Math refactoring (validated ~4e-3 rel err vs reference, gate is 2e-2):
  Softmax over keys is invariant to per-query constants, so the 192-dim
  q/k embedding contraction folds to a 128-dim one plus a per-key bias:
    scores[q,m] ~ qT_q . (BD qT_m)   with the per-key bias folded into
  Vaug[m] = [w_m * v_m, w_m], w_m = exp(c2.s_m); out = acc[:64]/acc[64].

Host-side prep (numpy, O(B*N) work only -- the O(B*N^2) attention stays
fully on device):
  - Lorentz normalization; qT [B,128,N] feature-major.
  - kT = BD^T @ qT and Vaug (device layout [128, NB, 66]) precomputed on
    host so the device runs nothing but the N^2 pipeline.
  - Final division by the denominator row + transpose to [B,N,16,4].
  - Rare overflow rows (per-query score max > ~70, from near-null
    Lorentz vectors; ~27 of 65536 queries) are recomputed exactly on the
    host: the device denominator acc[64,q] >= 0.88*max_k P[k,q] is a
    complete witness, so den < 1e25 proves a column had no overflow.

Device structure per batch (per core):
  - Scores TRANSPOSED per 128-key chunk (f32r = tf32-like PE mode,
    1 cycle/row):  ST[kc][k,q] = kT[:,kc]^T @ qT  -- P^T lands directly
    in SBUF, no DMA transpose, and softmax needs NO row-max machinery:
    P = exp(S - 12) in f32 range for all but the host-patched rows.
  - PV: accT[66,q] += Vaug[kc]^T @ PT[kc], one PSUM accumulation group
    across the 8 key chunks; denominator from Vaug's w-column.
  - Flat cross-batch software pipeline over units u = b*8+kc with ST+exp
    running 2 units ahead of PV; ACT does only Exp (one table load).
"""

import numpy as np

B, N = 64, 1024
NCORES = 8
BL = B // NCORES          # batches per core
NB = N // 128             # 128-key chunks per sequence
SCALE = 1.0 / np.sqrt(192.0)
EXP_BIAS = -12.0          # constant shift inside exp; cancels in division

_CACHE = {}


def _build_program():
    import concourse.bacc as bacc
    import concourse.tile as tile
    from concourse import mybir

    f32 = mybir.dt.float32

    nc = bacc.Bacc("TRN2", target_bir_lowering=False,
                   debug=False, num_devices=NCORES)

    aps = {
        "qkv": nc.dram_tensor("qkv", [BL, 128, 2 * N + NB * 66], f32,
                              kind="ExternalInput").ap(),
        "acc": nc.dram_tensor("acc", [BL, 66, N], f32,
                              kind="ExternalOutput").ap(),
    }

    with tile.TileContext(nc) as tc:
        _emit(tc, aps)

    nc.compile()
    return nc


def _emit(tc, aps):
    from contextlib import ExitStack
    from concourse import mybir

    nc = tc.nc
    f32 = mybir.dt.float32
    f32r = mybir.dt.float32r
    PS = "PSUM"
    Act = mybir.ActivationFunctionType

    qkv_d, acc_d = aps["qkv"], aps["acc"]

    with ExitStack() as ctx:
        singles = ctx.enter_context(tc.tile_pool(name="singles", bufs=1))
        qpool = ctx.enter_context(tc.tile_pool(name="qpool", bufs=BL))
        ptpool = ctx.enter_context(tc.tile_pool(name="ptpool", bufs=4))
        sbacc = ctx.enter_context(tc.tile_pool(name="sbacc", bufs=2))
        # PSUM budget (8 banks): ST ring 3x2 + accT 1x2
        psS = ctx.enter_context(tc.tile_pool(name="psS", bufs=3, space=PS))
        psAcc = ctx.enter_context(tc.tile_pool(name="psAcc", bufs=1, space=PS))

        ebias = singles.tile([128, 1], f32)
        nc.gpsimd.memset(ebias[:], EXP_BIAS)
        # warm the ACT exp table while the input DMAs run (the implicit
        # table load is 1.3us and would otherwise sit on the critical path)
        warm = singles.tile([128, 1], f32)
        nc.scalar.activation(out=warm[:], in_=ebias[:], func=Act.Exp)
        scratch = singles.tile([128, 512], f32)
        nc.gpsimd.memset(scratch[:], 0.0)

        # All batches' inputs DMA'd up front, one fused DMA per batch
        # (qT | kT | vaugT side by side). Batch 0 goes alone on the Pool
        # issue queue (its completion gates the first matmul); the rest go
        # on the otherwise-idle SP queue so their issue latency doesn't
        # delay batch 0's semaphore.
        qts, kts, vas = [], [], []
        for b in range(BL):
            qkv = qpool.tile([128, 2 * N + NB * 66], f32r, tag="qkv",
                             name=f"qkv{b}")
            eng = nc.gpsimd if b == 0 else nc.sync
            eng.dma_start(out=qkv[:], in_=qkv_d[b].bitcast(f32r))
            qts.append(qkv[:, 0:N])
            kts.append(qkv[:, N:2 * N])
            vas.append(qkv[:, 2 * N:].rearrange("p (c f) -> p c f", f=66))

        # PE p-state warm-up: ~3us of throwaway matmuls during the DMA
        # wait so the first real scores run at full clock.
        pwarm = psS.tile([128, 512], f32, tag="ST", name="pwarm")
        for _ in range(4):
            nc.tensor.matmul(pwarm[0:2, 0:256], scratch[:, 0:2],
                             scratch[:, 0:256], start=True, stop=True)

        accTs, pts = {}, {}

        def st_exp(u):
            b, kc = divmod(u, NB)
            ST = psS.tile([128, N], f32, tag="ST", name=f"ST{b}_{kc}")
            for h in range(2):
                cs = slice(h * 512, (h + 1) * 512)
                nc.tensor.matmul(ST[:, cs],
                                 kts[b][:, kc * 128:(kc + 1) * 128],
                                 qts[b][:, cs], start=True, stop=True)
            pt = ptpool.tile([128, N], f32r, tag="pt", name=f"pt{b}_{kc}")
            nc.scalar.activation(out=pt[:], in_=ST[:], func=Act.Exp,
                                 bias=ebias[:])
            pts[u] = pt

        def pv_unit(u):
            b, kc = divmod(u, NB)
            if kc == 0:
                accTs[b] = psAcc.tile([66, N], f32, tag="accT",
                                      name=f"accT{b}")
            accT = accTs[b]
            pt = pts.pop(u)
            for h in range(2):
                cs = slice(h * 512, (h + 1) * 512)
                nc.tensor.matmul(accT[:, cs], vas[b][:, kc, :], pt[:, cs],
                                 start=(kc == 0), stop=(kc == NB - 1))
            if kc == NB - 1:
                # epilogue split in halves so the copy/DMA overlaps the
                # next batch's first PVs (and shortens the final drain)
                accsb = sbacc.tile([66, N], f32, tag="accsb",
                                   name=f"accsb{b}")
                accT = accTs.pop(b)
                for h in range(2):
                    cs = slice(h * 512, (h + 1) * 512)
                    nc.vector.tensor_copy(accsb[:, cs], accT[:, cs])
                    nc.gpsimd.dma_start(out=acc_d[b, :, cs],
                                        in_=accsb[:, cs])

        NU = BL * NB
        st_exp(0)
        st_exp(1)
        for u in range(NU):
            if u + 2 < NU:
                st_exp(u + 2)
            pv_unit(u)


def _host_prepare(vectors, scalars, Wq, Wq_s, bq_s, Wk, Wk_s, bk_s, Wv):
    """Fold weights and precompute qT, kT, VaugT (O(B*N) work, f64->f32
    for the tiny weight folds, f32 sgemm for kT)."""
    METRIC = np.array([1.0, -1.0, -1.0, -1.0], dtype=np.float64)
    G = Wq.astype(np.float64).T @ Wk.astype(np.float64)            # [16,16]
    BD = np.zeros((128, 128), dtype=np.float64)
    for k in range(4):
        BD[k:64:4, k:64:4] = SCALE * METRIC[k] * G.T
    BD[64:, 64:] = SCALE * (Wk_s.astype(np.float64).T @ Wq_s.astype(np.float64))
    E = np.exp(Wv.astype(np.float64)).astype(np.float32)           # [16,16]
    c2s = (SCALE * (Wk_s.astype(np.float64).T @ bq_s.astype(np.float64))
           ).astype(np.float32)                                    # [64]
    BD32 = BD.astype(np.float32)

    v = np.asarray(vectors, dtype=np.float32)
    s = np.asarray(scalars, dtype=np.float32)
    sq = v * v
    nrm = sq[..., 0] - sq[..., 1] - sq[..., 2] - sq[..., 3]
    vecs = v / np.sqrt(np.clip(np.abs(nrm), 1e-5, None))[..., None]

    qT = np.empty((B, 128, N), dtype=np.float32)
    qT[:, 0:64, :] = vecs.reshape(B, N, 64).transpose(0, 2, 1)
    qT[:, 64:128, :] = s.transpose(0, 2, 1)
    kT = np.einsum('de,bdn->ben', BD32, qT)                        # BD^T @ qT

    w = np.exp(s @ c2s)[..., None]                                 # [B,N,1]
    vv = np.einsum('ij,bnjk->bnik', E, vecs).reshape(B, N, 64)
    vaug = np.concatenate([vv * w, w, np.ones_like(w)], axis=2)    # [B,N,66]
    vaugT = np.ascontiguousarray(
        vaug.reshape(B, NB, 128, 66).transpose(0, 2, 1, 3))        # [B,128,NB,66]

    qkv = np.concatenate([qT, kT, vaugT.reshape(B, 128, NB * 66)], axis=2)
    in_maps = []
    for c in range(NCORES):
        sl = slice(c * BL, (c + 1) * BL)
        in_maps.append({"qkv": np.ascontiguousarray(qkv[sl])})
    return in_maps


def _prepare_in_maps(vectors, scalars, Wq, Wq_s, bq_s, Wk, Wk_s, bk_s, Wv):
    return _host_prepare(vectors, scalars, Wq, Wq_s, bq_s, Wk, Wk_s, bk_s, Wv)


def _run(in_maps, **kw):
    from concourse.bass_utils import run_bass_kernel_spmd
    nc = _get_program()
    return run_bass_kernel_spmd(nc, in_maps, list(range(NCORES)), **kw)


def _get_program():
    if "nc" not in _CACHE:
        _CACHE["nc"] = _build_program()
    return _CACHE["nc"]


def _patch_rows(out, bad, vectors, scalars, Wq, Wq_s, bq_s, Wk, Wk_s, bk_s,
                Wv):
    """Recompute flagged query rows exactly (f64 reference math).

    The device skips per-query max subtraction; exp(S-12) can overflow
    f32 for the rare queries whose row max exceeds ~70 (near-null
    Lorentz vectors give normalized entries up to ~415 and scores up to
    ~915). den >= 0.88 * P_max, so den < 1e25 proves no overflow.
    """
    METRIC = np.array([1.0, -1.0, -1.0, -1.0])
    for b in np.nonzero(bad.any(axis=1))[0]:
        v = vectors[b].astype(np.float64)
        s = scalars[b].astype(np.float64)
        nrm = np.einsum('nik,k->ni', v * v, METRIC)[..., None]
        vecs = v / np.sqrt(np.clip(np.abs(nrm), 1e-5, None))
        k_v = np.einsum('ij,njk->nik', Wk.astype(np.float64), vecs)
        k_s = s @ Wk_s.astype(np.float64).T + bk_s.astype(np.float64)
        k = np.concatenate([(k_v * METRIC).reshape(N, -1), k_s], axis=-1)
        vv = np.einsum('ij,njk->nik', np.exp(Wv.astype(np.float64)),
                       vecs).reshape(N, -1)
        rows = np.nonzero(bad[b])[0]
        q_v = np.einsum('ij,njk->nik', Wq.astype(np.float64), vecs[rows])
        q_s = s[rows] @ Wq_s.astype(np.float64).T + bq_s.astype(np.float64)
        q = np.concatenate([q_v.reshape(len(rows), -1), q_s], axis=-1)
        S = (q @ k.T) / np.sqrt(192.0)
        S -= S.max(axis=1, keepdims=True)
        P = np.exp(S)
        out[b, rows] = ((P @ vv) / P.sum(axis=1, keepdims=True)).astype(
            np.float32)


def kernel(vectors, scalars, Wq, Wq_s, bq_s, Wk, Wk_s, bk_s, Wv):
    args = [np.asarray(a, dtype=np.float32) for a in
            (vectors, scalars, Wq, Wq_s, bq_s, Wk, Wk_s, bk_s, Wv)]
    in_maps = _prepare_in_maps(*args)
    res = _run(in_maps)
    acc = np.concatenate([res.results[c]["acc"] for c in range(NCORES)],
                         axis=0)                     # [B, 66, N]
    den = acc[:, 64, :]
    with np.errstate(over="ignore", invalid="ignore", divide="ignore"):
        out = (acc[:, 0:64, :] / acc[:, 64:65, :]).transpose(0, 2, 1)
    bad = (~np.isfinite(den)) | (den >= 1e25) | (
        ~np.isfinite(out).all(axis=2))               # [B, N]
    if bad.any():
        _patch_rows(out, bad, *args)
    return np.ascontiguousarray(out.reshape(B, N, 16, 4), dtype=np.float32)
